# revision 11
# baseline (speedup 1.0000x reference)
"""Causal self-attention (B=4, T=2048, E=512, H=8) on 8 TRN2 NeuronCores.

Sharding: core c -> (batch b = c//2, head-group hg = c%2, 4 heads each).
Host sums the two partial projection outputs per batch.

Design (v2):
- qkv proj per token group tg; q/k feature-major (qkT), v token-major + ones
  column (v4) for softmax denominators.
- Scores as S^T = K^T.T-style matmuls with K=64 contraction; the two heads of
  a pair sit at partitions 0-63 / 64-127 so their score matmuls occupy
  disjoint PE row groups and run concurrently.
- exp split between ScalarE (activation Exp) and VectorE (Schraudolph fp16
  bit-trick: i16 = round(x*1477.32 + 15301), bitcast to f16; ~3% per-element,
  validated < 2e-2 end-to-end).
- PV with v stationary (LDWEIGHTS 65 cols, hidden): yraw^T[65, 512] per
  (head, qg) in PSUM; row 64 = softmax denominator.
- Normalize: denom -> f16 sbuf -> K=1 ones-matmul broadcast across 64
  partitions -> reciprocal_approx_fast -> multiply into yT (no transposes).
- Output proj: 4 accumulating K=64 matmuls per token chunk (even/odd head
  yT tiles); bias pre-broadcast into sbuf once (also absorbs the v-proj bias,
  folded on host into bpp = bp + bv @ Wp).
"""

from contextlib import ExitStack

import numpy as np

import concourse.bass as bass
import concourse.mybir as mybir
import concourse.tile as tile
from concourse import bacc
from concourse.bass import ts
from concourse.bass_utils import run_bass_kernel_spmd

f32 = mybir.dt.float32
f16 = mybir.dt.float16
i16 = mybir.dt.int16
FA = mybir.ActivationFunctionType
MUL = mybir.AluOpType.mult
ADD = mybir.AluOpType.add

B, T, E = 4, 2048, 512
H, D = 8, 64
HPC = 4              # heads per core
EC = HPC * D         # 256
P = 128
NCORES = 8
TQ = T // P          # 16 token chunks
NQG = T // 512       # 4 query groups
EO = E // P          # 4 contraction subtiles
SCALE = 1.0 / np.sqrt(D)

# Schraudolph fp16 fast-exp constants (round-half-even on DVE f32->i16)
A16 = float(2.0**10 / np.log(2.0))
B16 = 15360.0 - 59.0

# dve8: of every 8 exp tiles, this many go to the DVE (rest ScalarE)
CFG = {"dve8": 4, "pS_bufs": 2, "pG_bufs": 2, "expS_bufs": 6, "xT_bufs": 2, "no_ilv": 0}


def _emit(tc, ctx, aps, reps=1):
    nc = tc.nc
    z = aps["z"]

    cst = ctx.enter_context(tc.tile_pool(name="cst", bufs=1))
    wqk_sb = cst.tile([P, EO, 2 * EC], f16)
    for eo in range(EO):
        nc.sync.dma_start(wqk_sb[:, eo, :], aps["wqk"][:, eo, :])
    bqk_sb = cst.tile([P, 4], f32)
    nc.sync.dma_start(bqk_sb, aps["bqk"])
    wv_sb = cst.tile([P, EO, EC], f16)
    nc.sync.dma_start(wv_sb, aps["wv"])
    wpe_sb = cst.tile([64, 2, E], f16)
    nc.sync.dma_start(wpe_sb, aps["wpe"])
    wpo_sb = cst.tile([64, 2, E], f16)
    nc.sync.dma_start(wpo_sb, aps["wpo"])
    bpp_sb = cst.tile([1, E], f16)
    nc.sync.dma_start(bpp_sb, aps["bpp"])
    ones_sb = cst.tile([1, P], f16)
    nc.sync.dma_start(ones_sb, aps["ones"])
    mask_sb = cst.tile([P, 4, 512], f16)
    nc.sync.dma_start(mask_sb, aps["mask"])
    ones2 = cst.tile([P, 64], f16)
    nc.vector.memset(ones2, 1.0)

    big = ctx.enter_context(tc.tile_pool(name="big", bufs=1))
    qkT = big.tile([P, EO, T], f16)          # sub 0-1: q^T, 2-3: k^T
    v_sb = big.tile([P, TQ, HPC * 65], f16)  # per head: 64 v cols + ones col
    v4 = v_sb.rearrange("p t (h c) -> p t h c", c=65)
    yTe = big.tile([64, 2, T], f16)          # heads 0, 2 (partitions 0-63)
    yTo = big.tile([64, 2, T], f16)          # heads 1, 3
    bp_bc = big.tile([P, E], f16)            # proj bias broadcast to 128 rows

    xTp = ctx.enter_context(tc.tile_pool(name="xTp", bufs=CFG["xT_bufs"]))
    pS = ctx.enter_context(tc.tile_pool(name="pS", bufs=CFG["pS_bufs"], space="PSUM"))
    pV = ctx.enter_context(tc.tile_pool(name="pV", bufs=1, space="PSUM"))
    pG = ctx.enter_context(tc.tile_pool(name="pG", bufs=CFG["pG_bufs"], space="PSUM"))
    expSp = ctx.enter_context(tc.tile_pool(name="expSp", bufs=CFG["expS_bufs"]))
    denp = ctx.enter_context(tc.tile_pool(name="denp", bufs=2))
    rcbp = ctx.enter_context(tc.tile_pool(name="rcbp", bufs=2))
    zout = ctx.enter_context(tc.tile_pool(name="zout", bufs=3))

    # one-time: broadcast proj bias to all 128 partitions
    pb = pG.tile([P, E], f32, tag="g")
    nc.tensor.matmul(pb, lhsT=ones_sb, rhs=bpp_sb, start=True, stop=True)
    nc.vector.tensor_copy(bp_bc, pb)

    exp_ctr = [0]

    def emit_exp(pSt, out_ap):
        """exp of one [128, n*512] psum tile into expS f16, engine by knob."""
        use_dve = (exp_ctr[0] % 8) < CFG["dve8"]
        exp_ctr[0] += 1
        if use_dve:
            nc.vector.tensor_scalar(
                out_ap.bitcast(i16), pSt, A16, B16, MUL, ADD
            )
        else:
            nc.scalar.activation(out_ap, pSt, FA.Exp)

    def phase1(xT, tg):
        """qkv projection for token group tg (tokens tg*512 .. +512)."""
        for j in range(4):
            jc = j  # q chunks 0,1 then k chunks 2,3
            pq = pG.tile([P, 512], f32, tag="g")
            for eo in range(EO):
                nc.tensor.matmul(
                    pq,
                    lhsT=wqk_sb[:, eo, ts(jc, P)],
                    rhs=xT[:, eo, ts(tg, 512)],
                    start=(eo == 0),
                    stop=(eo == EO - 1),
                )
            nc.scalar.activation(
                qkT[:, jc, ts(tg, 512)], pq, FA.Identity, bias=bqk_sb[:, jc : jc + 1]
            )
            tq = 4 * tg + j
            pv = pG.tile([P, 512], f32, tag="g")
            for eo in range(EO):
                nc.tensor.matmul(
                    pv[:, :EC],
                    lhsT=xT[:, eo, ts(tq, P)],
                    rhs=wv_sb[:, eo, :],
                    start=(eo == 0),
                    stop=(eo == EO - 1),
                )
            nc.scalar.activation(
                v4[:, tq, :, 0:64],
                pv[:, :EC].rearrange("p (h c) -> p h c", c=64),
                FA.Copy,
            )

    def scores_chunks(qg, expS4):
        """Generator: S^T block-pairs + exp for all 4 heads; yields between
        chunks so PE-dense work can interleave. expS4 = 4 expS tiles."""
        nb = 4 * qg + 4
        for pair in range(2):
            heads = (2 * pair, 2 * pair + 1)
            q_sub = pair
            k_sub = 2 + pair
            for g0 in range(0, nb, 2):
                tiles = []
                for h in heads:
                    hp = (h % 2) * 64
                    pSt = pS.tile([P, 2, 512], f32, tag="s", name=f"pS_{qg}_{h}_{g0}")
                    tiles.append(pSt)
                    for kk in range(2):
                        kb = g0 + kk
                        nc.tensor.matmul(
                            pSt[:, kk, :],
                            lhsT=qkT[hp : hp + 64, k_sub, ts(kb, P)],
                            rhs=qkT[hp : hp + 64, q_sub, ts(qg, 512)],
                            start=True,
                            stop=True,
                        )
                for j, h in enumerate(heads):
                    emit_exp(tiles[j], expS4[2 * pair + j][:, g0 : g0 + 2, :])
                yield
            # causal mask on the 4 diagonal blocks of each head
            for j in range(2):
                nc.vector.tensor_tensor(
                    expS4[2 * pair + j][:, 4 * qg : 4 * qg + 4, :],
                    expS4[2 * pair + j][:, 4 * qg : 4 * qg + 4, :],
                    mask_sb,
                    MUL,
                )
            yield

    def pv_pair(qg, pair, expS4, sgen):
        """PV for the two heads of `pair` -> pv2 psum [65, 2, 512].
        Pulls one scores chunk from sgen per kb step to interleave."""
        nb = 4 * qg + 4
        heads = (2 * pair, 2 * pair + 1)
        pv2 = pV.tile([65, 2, 512], f32, tag="v", name=f"pv2_{qg}_{pair}")
        for kb in range(nb):
            for j, h in enumerate(heads):
                nc.tensor.matmul(
                    pv2[:, j, :],
                    lhsT=v4[:, kb, h, :],
                    rhs=expS4[2 * pair + j][:, kb, :],
                    start=(kb == 0),
                    stop=(kb == nb - 1),
                )
            _pull(sgen)
        return pv2

    def normalize(qg, pair, pv2):
        """denom -> bcast -> reciprocal -> scale into yT tiles."""
        heads = (2 * pair, 2 * pair + 1)
        den = denp.tile([P, 2, 512], f16, tag="d")
        nc.vector.tensor_copy(den[64:65, :, :], pv2[64:65, :, :])
        for j, h in enumerate(heads):
            bc = pG.tile([P, 512], f32, tag="g")
            nc.tensor.matmul(
                bc[0:64, :],
                lhsT=ones2[64:65, :],
                rhs=den[64:65, j, :],
                start=True,
                stop=True,
            )
            rcb = rcbp.tile([64, 512], f32, tag="r")
            nc.vector.reciprocal_approx_fast(rcb, bc[0:64, :])
            yT = yTe if h % 2 == 0 else yTo
            nc.vector.tensor_tensor(
                yT[:, h // 2, ts(qg, 512)], pv2[0:64, j, :], rcb, MUL
            )

    def proj_z(qg, sgen):
        """output projection for the 4 token chunks of query group qg."""
        for tq in range(4 * qg, 4 * qg + 4):
            pz = pG.tile([P, 512], f32, tag="g", name=f"pz_{tq}")
            k = 0
            for yT, wp in ((yTe, wpe_sb), (yTo, wpo_sb)):
                for e in range(2):
                    nc.tensor.matmul(
                        pz,
                        lhsT=yT[:, e, ts(tq, P)],
                        rhs=wp[:, e, :],
                        start=(k == 0),
                        stop=(k == 3),
                    )
                    k += 1
            zt = zout.tile([P, E], f16, tag="z", name=f"zt_{tq}")
            nc.vector.tensor_tensor(zt, pz, bp_bc, ADD)
            nc.sync.dma_start(z[ts(tq, P), :], zt)
            _pull(sgen)

    def _pull(gen):
        if gen is not None:
            try:
                next(gen)
            except StopIteration:
                pass

    def _drain(gen):
        if gen is not None:
            for _ in gen:
                pass

    def new_exp4(qg):
        return [
            expSp.tile([P, TQ, 512], f16, tag="e", name=f"exp{j}_{qg}")
            for j in range(4)
        ]

    nc.vector.memset(v4[:, :, :, 64], 1.0)
    for _ in range(reps):
        xT = xTp.tile([P, EO, T], f16, tag="x")
        for eo in range(0, EO, 2):
            for th in range(4):
                nc.sync.dma_start(
                    xT[:, eo : eo + 2, ts(th, T // 4)],
                    aps["xT"][:, eo : eo + 2, ts(th, T // 4)],
                )
        phase1(xT, 0)
        exp4 = new_exp4(0)
        _drain(scores_chunks(0, exp4))
        for qg in range(NQG):
            # dense PE work for qg, interleaved with scores+exp of qg+1
            pv2a = pv_pair(qg, 0, exp4, None)
            normalize(qg, 0, pv2a)
            if qg < NQG - 1:
                phase1(xT, qg + 1)
                exp4n = new_exp4(qg + 1)
                sgen = scores_chunks(qg + 1, exp4n)
                if CFG["no_ilv"]:
                    _drain(sgen)
                    sgen = None
            else:
                exp4n, sgen = None, None
            pv2b = pv_pair(qg, 1, exp4, sgen)
            normalize(qg, 1, pv2b)
            if qg > 0:
                proj_z(qg - 1, sgen)
            _drain(sgen)
            exp4 = exp4n
        proj_z(NQG - 1, None)


def build(reps=1):
    nc = bacc.Bacc("TRN2", target_bir_lowering=False, debug=False)
    aps = {
        "xT": nc.dram_tensor("xT", [P, EO, T], f16, kind="ExternalInput").ap(),
        "wqk": nc.dram_tensor("wqk", [P, EO, 2 * EC], f16, kind="ExternalInput").ap(),
        "bqk": nc.dram_tensor("bqk", [P, 4], f32, kind="ExternalInput").ap(),
        "wv": nc.dram_tensor("wv", [P, EO, EC], f16, kind="ExternalInput").ap(),
        "wpe": nc.dram_tensor("wpe", [64, 2, E], f16, kind="ExternalInput").ap(),
        "wpo": nc.dram_tensor("wpo", [64, 2, E], f16, kind="ExternalInput").ap(),
        "bpp": nc.dram_tensor("bpp", [1, E], f16, kind="ExternalInput").ap(),
        "ones": nc.dram_tensor("ones", [1, P], f16, kind="ExternalInput").ap(),
        "mask": nc.dram_tensor("mask", [P, 4, 512], f16, kind="ExternalInput").ap(),
        "z": nc.dram_tensor("z", [T, E], f16, kind="ExternalOutput").ap(),
    }
    with tile.TileContext(nc) as tc, ExitStack() as ctx:
        _emit(tc, ctx, aps, reps=reps)
    nc.compile()
    return nc


def make_in_maps(x, c_attn_w, c_attn_b, c_proj_w, c_proj_b):
    x = np.asarray(x, np.float32)
    W = np.asarray(c_attn_w, np.float32)
    bW = np.asarray(c_attn_b, np.float32)
    Wp = np.asarray(c_proj_w, np.float32)
    bp = np.asarray(c_proj_b, np.float32)

    ones = np.ones((1, P), np.float16)
    # mask[p, j, c] = 1 iff query col c >= key row p + 128*j (causal staircase)
    pp = np.arange(P)[:, None, None]
    jj = np.arange(4)[None, :, None]
    cc = np.arange(512)[None, None, :]
    mask = (cc >= pp + 128 * jj).astype(np.float16)
    in_maps = []
    for c in range(NCORES):
        b, hg = c // 2, c % 2
        qs = slice(hg * EC, (hg + 1) * EC)
        ks = slice(E + hg * EC, E + (hg + 1) * EC)
        vs = slice(2 * E + hg * EC, 2 * E + (hg + 1) * EC)
        wqk = np.concatenate([W[:, qs] * SCALE, W[:, ks]], axis=1)  # [512, 512]
        bqk = np.concatenate([bW[qs] * SCALE, bW[ks]])              # [512]
        xT = np.ascontiguousarray(
            x[b].T.reshape(EO, P, T).transpose(1, 0, 2)
        ).astype(np.float16)
        Wp_core = Wp[hg * EC : (hg + 1) * EC, :]                    # [256, 512]
        wpe = np.stack([Wp_core[0:64], Wp_core[128:192]], axis=1)   # [64, 2, 512]
        wpo = np.stack([Wp_core[64:128], Wp_core[192:256]], axis=1)
        bpp = (bp if hg == 0 else np.zeros_like(bp)) + bW[vs] @ Wp_core
        in_maps.append({
            "xT": xT,
            "wqk": np.ascontiguousarray(
                wqk.reshape(EO, P, 2 * EC).transpose(1, 0, 2)
            ).astype(np.float16),
            "bqk": np.ascontiguousarray(bqk.reshape(4, P).T),
            "wv": np.ascontiguousarray(
                W[:, vs].reshape(EO, P, EC).transpose(1, 0, 2)
            ).astype(np.float16),
            "wpe": np.ascontiguousarray(wpe).astype(np.float16),
            "wpo": np.ascontiguousarray(wpo).astype(np.float16),
            "bpp": bpp[None].astype(np.float16),
            "ones": ones,
            "mask": mask,
        })
    return in_maps


_NC_CACHE = {}


def kernel(x, c_attn_w, c_attn_b, c_proj_w, c_proj_b):
    if "nc" not in _NC_CACHE:
        _NC_CACHE["nc"] = build()
    nc = _NC_CACHE["nc"]
    in_maps = make_in_maps(x, c_attn_w, c_attn_b, c_proj_w, c_proj_b)
    res = run_bass_kernel_spmd(nc, in_maps, core_ids=list(range(NCORES)))
    out = np.empty((B, T, E), np.float32)
    for b in range(B):
        out[b] = (
            res.results[2 * b]["z"].astype(np.float32)
            + res.results[2 * b + 1]["z"].astype(np.float32)
        )
    return out
